# revision 14
# baseline (speedup 1.0000x reference)
"""Recursive LSTM decoder (T=512, B=512, I=128, H=512) on 8 trn2 NeuronCores.

Strategy: data-parallel over batch (64 rows/core, weights replicated, no
collectives). All on-chip state is kept in transposed layout
[feature-on-partition, batch-on-free] so the serial recurrence needs no
transposes. Matmul inputs are bf16 (1 cycle/row on PE), accumulation and
elementwise math are fp32; the cell state c stays fp32.

Per step (per core):
  gates.T[m-chunk 128, b 64] = sum_k Wcat.T-chunk(k,m) @ catT-chunk(k)
    (16 m-chunks x 5 k-chunks; PSUM grouped by output H-chunk so ACT/DVE of
     group c overlaps PE of group c+1)
  i,f,o = sigmoid(. + b), g = tanh(. + b)   (bias folded into ACT)
  c = f*c + i*g ; h = o*tanh(c)
  feedback: inT = tanh(0.5*(fcW.T-chunks @ hT) + fc_b/2)   [= 2*sigmoid(z)-1]
  output:   out[64,128] = tanh(0.5*(hT-chunks as stationary @ fcW-moving + fc_b))
  out -> DRAM at index (T-1-t)  (reference stores outputs reversed)

All constants+init are shipped in 2 bundled DMAs and the output store uses
the single SWDGE queue: per-instruction sync-wait fan-in must stay <= the
ISA cap (walrus "Too many sync wait commands" otherwise).
"""

import numpy as np
import ml_dtypes

import concourse.bass as bass
import concourse.mybir as mybir
import concourse.tile as tile
from concourse import bacc
from concourse.bass import ds
from concourse.bass_utils import run_bass_kernel_spmd

T, B, I, H = 512, 512, 128, 512
NCORES = 8
BS = B // NCORES          # 64 batch rows per core
HC = H // 128             # 4 h chunks
NM = (4 * H) // 128       # 16 gate m-chunks
NK = (I + H) // 128       # 5 cat k-chunks (1 input + 4 hidden)

# bf16 constant-bundle column offsets
OFF_WG = 0                       # [128, NM*NK*128] gate weight chunks
OFF_WFC = OFF_WG + NM * NK * 128  # [128, HC*128] fc weight chunks
OFF_XT = OFF_WFC + HC * 128      # [128, BS] x[T-1] transposed
OFF_H0 = OFF_XT + BS             # [128, HC*BS] h0 transposed
OFF_FCBR = OFF_H0 + HC * BS      # [1, 128] fc bias row (row 0 only)
CB_COLS = OFF_FCBR + 128
# f32 constant-bundle column offsets
OFF_BG = 0                       # [128, NM] fused gate bias
OFF_FCBH = OFF_BG + NM           # [128, 1] fc_b / 2
OFF_C0 = OFF_FCBH + 1            # [128, HC*BS] c0 transposed
CF_COLS = OFF_C0 + HC * BS

BF16 = mybir.dt.bfloat16
F32 = mybir.dt.float32
AF = mybir.ActivationFunctionType


def build(nsteps: int):
    nc = bacc.Bacc()
    cb16 = nc.dram_tensor("cb16", [128, CB_COLS], BF16, kind="ExternalInput")
    cf32 = nc.dram_tensor("cf32", [128, CF_COLS], F32, kind="ExternalInput")
    idx0 = nc.dram_tensor("idx0", [BS, 1], mybir.dt.int32, kind="ExternalInput")
    out = nc.dram_tensor("out", [nsteps * BS, I], F32, kind="ExternalOutput")

    with tile.TileContext(nc) as tc:
        with (
            tc.tile_pool(name="consts", bufs=1) as consts,
            tc.tile_pool(name="state", bufs=1) as state,
            tc.tile_pool(name="gact", bufs=3) as gact,
            tc.tile_pool(name="hwork", bufs=2) as hwork,
            tc.tile_pool(name="outp", bufs=3) as outp,
            tc.tile_pool(name="pg", bufs=4, space="PSUM") as pgp,
            tc.tile_pool(name="pf", bufs=2, space="PSUM") as pfp,
            tc.tile_pool(name="po", bufs=2, space="PSUM") as pop,
        ):
            CB = consts.tile([128, CB_COLS], BF16)
            nc.sync.dma_start(out=CB, in_=cb16[:])
            CF = consts.tile([128, CF_COLS], F32)
            nc.sync.dma_start(out=CF, in_=cf32[:])
            ones = consts.tile([1, BS], BF16)
            nc.vector.memset(ones, 1.0)

            def wg_chunk(m, k):
                o = OFF_WG + (m * NK + k) * 128
                return CB[:, o:o + 128]

            def wfc_chunk(k):
                o = OFF_WFC + k * 128
                return CB[:, o:o + 128]

            fb_r = CB[0:1, OFF_FCBR:OFF_FCBR + 128]
            bgs = CF[:, OFF_BG:OFF_BG + NM]
            fb_h = CF[:, OFF_FCBH:OFF_FCBH + 1]

            hT = state.tile([128, HC, BS], BF16)
            nc.vector.tensor_copy(
                hT, CB[:, OFF_H0:OFF_H0 + HC * BS].rearrange(
                    "p (c b) -> p c b", c=HC))
            cT = state.tile([128, HC, BS], F32)
            nc.vector.tensor_copy(
                cT, CF[:, OFF_C0:OFF_C0 + HC * BS].rearrange(
                    "p (c b) -> p c b", c=HC))
            inT = state.tile([128, BS], BF16)
            nc.vector.tensor_copy(inT, CB[:, OFF_XT:OFF_XT + BS])
            # scatter row indices for the reversed output store; decremented
            # by BS each step so no dynamic DMA addressing is needed
            idx = state.tile([BS, 1], mybir.dt.int32)
            nc.sync.dma_start(out=idx, in_=idx0[:])

            def step(t):
                # gates + cell update, one H-chunk group at a time
                hnew = hwork.tile([128, HC, BS], BF16, tag="hnew")
                for c in range(HC):
                    pg = pgp.tile([128, 4, BS], F32, tag="pg")
                    for j, m in enumerate((c, 4 + c, 8 + c, 12 + c)):
                        for k in range(NK):
                            mv = inT if k == 0 else hT[:, k - 1, :]
                            nc.tensor.matmul(
                                pg[:, j, :],
                                lhsT=wg_chunk(m, k),
                                rhs=mv,
                                start=(k == 0),
                                stop=(k == NK - 1),
                            )
                    i_s = gact.tile([128, BS], F32, tag="i_s")
                    f_s = gact.tile([128, BS], F32, tag="f_s")
                    g_s = gact.tile([128, BS], F32, tag="g_s")
                    o_s = gact.tile([128, BS], F32, tag="o_s")
                    nc.scalar.activation(i_s, pg[:, 0, :], AF.Sigmoid,
                                         bias=bgs[:, c:c + 1])
                    nc.scalar.activation(f_s, pg[:, 1, :], AF.Sigmoid,
                                         bias=bgs[:, 4 + c:5 + c])
                    nc.scalar.activation(g_s, pg[:, 2, :], AF.Tanh,
                                         bias=bgs[:, 8 + c:9 + c])
                    nc.scalar.activation(o_s, pg[:, 3, :], AF.Sigmoid,
                                         bias=bgs[:, 12 + c:13 + c])
                    v_s = gact.tile([128, BS], F32, tag="v_s")
                    u_s = gact.tile([128, BS], F32, tag="u_s")
                    nc.vector.tensor_mul(v_s, i_s, g_s)
                    nc.vector.tensor_mul(u_s, f_s, cT[:, c, :])
                    nc.vector.tensor_add(cT[:, c, :], u_s, v_s)
                    tc_s = gact.tile([128, BS], F32, tag="tc_s")
                    nc.scalar.activation(tc_s, cT[:, c, :], AF.Tanh)
                    nc.vector.tensor_mul(hnew[:, c, :], o_s, tc_s)

                # feedback fc: inT = tanh(0.5*fc(h) + fc_b/2)  [128 i, BS b]
                pf = pfp.tile([128, BS], F32, tag="pf")
                for k in range(HC):
                    nc.tensor.matmul(pf, lhsT=wfc_chunk(k), rhs=hnew[:, k, :],
                                     start=(k == 0), stop=(k == HC - 1))
                nc.scalar.activation(inT, pf, AF.Tanh, bias=fb_h, scale=0.5)

                # output fc in [b, i] layout for clean DMA; bias via K=1 matmul
                po = pop.tile([BS, 128], F32, tag="po")
                for k in range(HC):
                    nc.tensor.matmul(po, lhsT=hnew[:, k, :], rhs=wfc_chunk(k),
                                     start=(k == 0), stop=False)
                nc.tensor.matmul(po, lhsT=ones, rhs=fb_r, start=False, stop=True)
                ob = outp.tile([BS, 128], F32, tag="ob")
                nc.scalar.activation(ob, po, AF.Tanh, scale=0.5)
                nc.gpsimd.indirect_dma_start(
                    out=out[:],
                    out_offset=bass.IndirectOffsetOnAxis(ap=idx[:, :1], axis=0),
                    in_=ob,
                    in_offset=None,
                )
                nc.vector.tensor_scalar_add(idx, idx, -BS)

                # commit h state for next step (after all reads of old hT)
                nc.vector.tensor_copy(hT, hnew)

            with tc.For_i(0, nsteps, 1, staggered_reset=True) as t:
                step(t)

    nc.finalize()
    return nc


_cache = {}


def _get_nc(nsteps):
    if nsteps not in _cache:
        _cache[nsteps] = build(nsteps)
    return _cache[nsteps]


def _prep_inputs(x, h0, c0, W_ih, W_hh, b_ih, b_hh, fc_W, fc_b, nsteps):
    f32 = np.float32
    bf16 = ml_dtypes.bfloat16
    x = np.asarray(x, f32)
    h0 = np.asarray(h0, f32)
    c0 = np.asarray(c0, f32)
    W_cat = np.concatenate([np.asarray(W_ih, f32), np.asarray(W_hh, f32)], axis=1)
    wg_np = W_cat.reshape(NM, 128, NK, 128).transpose(3, 0, 2, 1).reshape(
        128, NM * NK * 128)
    fc_W = np.asarray(fc_W, f32)
    wfc_np = fc_W.reshape(I, HC, 128).transpose(2, 1, 0).reshape(128, HC * 128)
    b = np.asarray(b_ih, f32) + np.asarray(b_hh, f32)
    bg_np = b.reshape(NM, 128).T
    fc_b = np.asarray(fc_b, f32)

    cf = np.zeros((128, CF_COLS), f32)
    cf[:, OFF_BG:OFF_BG + NM] = bg_np
    cf[:, OFF_FCBH] = 0.5 * fc_b

    cb_common = np.zeros((128, CB_COLS), f32)
    cb_common[:, OFF_WG:OFF_WG + NM * NK * 128] = wg_np
    cb_common[:, OFF_WFC:OFF_WFC + HC * 128] = wfc_np
    cb_common[0, OFF_FCBR:OFF_FCBR + 128] = fc_b

    in_maps = []
    for core in range(NCORES):
        sl = slice(core * BS, (core + 1) * BS)
        cb = cb_common.copy()
        cb[:, OFF_XT:OFF_XT + BS] = x[nsteps - 1, sl, :].T
        cb[:, OFF_H0:OFF_H0 + HC * BS] = \
            h0[0, sl, :].reshape(BS, HC, 128).transpose(2, 1, 0).reshape(128, -1)
        cfc = cf.copy()
        cfc[:, OFF_C0:OFF_C0 + HC * BS] = \
            c0[0, sl, :].reshape(BS, HC, 128).transpose(2, 1, 0).reshape(128, -1)
        idx_np = ((nsteps - 1) * BS + np.arange(BS, dtype=np.int32))[:, None]
        in_maps.append({
            "cb16": np.ascontiguousarray(cb).astype(bf16),
            "cf32": np.ascontiguousarray(cfc),
            "idx0": np.ascontiguousarray(idx_np),
        })
    return in_maps


def run(x, h0, c0, W_ih, W_hh, b_ih, b_hh, fc_W, fc_b, nsteps=T, **spmd_kwargs):
    nc = _get_nc(nsteps)
    in_maps = _prep_inputs(x, h0, c0, W_ih, W_hh, b_ih, b_hh, fc_W, fc_b, nsteps)
    res = run_bass_kernel_spmd(nc, in_maps, core_ids=list(range(NCORES)),
                               **spmd_kwargs)
    outs = np.concatenate(
        [r["out"].reshape(nsteps, BS, I) for r in res.results], axis=1
    )
    return outs, res


def kernel(x, enc_hiddens, h0, c0, W_ih, W_hh, b_ih, b_hh, fc_W, fc_b):
    outs, _ = run(x, h0, c0, W_ih, W_hh, b_ih, b_hh, fc_W, fc_b, nsteps=T)
    return outs.astype(np.float32)
